# revision 6
# baseline (speedup 1.0000x reference)
"""Trainium2 Bass kernel for nn_BiBoAttention (B=2, S=2048, D=2048, H=16).

Sharding: 8 cores = 2 batches x 4 head-groups (4 heads of 128 dims each).
Per core: QKV projection + RoPE + causal softmax attention + partial Wo.
Host sums the 4 partial outputs per batch.

Design (v2):
- All matmul inputs bf16 (f32 PSUM accumulation): same PE rate as f32r,
  half the SBUF/DMA, end-to-end rel err ~4e-3 vs the 2e-2 gate.
- Scores computed TRANSPOSED (S^T[k,q] = K-block @ Q^T): the probs come
  out in [k, q] layout, which is exactly what the PV matmul needs as its
  moving operand -- no P transposes, no PSUM->SBUF P copies.
- Softmax without max-subtraction (shift invariance; scores are O(1)
  for this problem so exp cannot overflow). Denominator l computed with
  free-dim-1 ones-matmuls on the PE (~free), inverted on DVE, spread
  back over q columns via PE transpose + GPSIMD partition_broadcast,
  and applied to O^T during the single PSUM->SBUF move.
- Everything stays in SBUF: no DRAM round-trips for Q/K/V/O.
- Phase-1 chunks (512 tokens), attention blocks, and the output
  projection are interleaved in emission order so the PE never starves.
"""
import math
import ml_dtypes
import numpy as np
from contextlib import ExitStack

import concourse.bass as bass
import concourse.mybir as mybir
import concourse.tile as tile
from concourse import bacc
from concourse.bass_utils import run_bass_kernel_spmd

F32 = mybir.dt.float32
BF16 = mybir.dt.bfloat16
ALU = mybir.AluOpType
ACTF = mybir.ActivationFunctionType

B = 2
D = 2048
H = 16
HD = 128
P = 128
FC = D // P          # 16 feature chunks
NH = 4               # heads per core
DG = NH * HD         # 512 group width
NCORES = 8
ROPE_THETA = 10000.0
TC = 512             # phase-1 token chunk (one q-block)


def build_program(S, mode):
    KQ = S // 512        # q blocks / token chunks
    NKB_ALL = S // P     # 128-token k-blocks
    nc = bacc.Bacc("TRN2", target_bir_lowering=False, debug=False,
                   num_devices=NCORES)

    xt_d = nc.declare_dram_parameter("xt", [P, FC, S], BF16, isOutput=False)
    wq_d = nc.declare_dram_parameter("wq", [P, FC, NH, HD], BF16, isOutput=False)
    wk_d = nc.declare_dram_parameter("wk", [P, FC, NH, HD], BF16, isOutput=False)
    wv_d = nc.declare_dram_parameter("wv", [P, FC, DG], BF16, isOutput=False)
    wo_d = nc.declare_dram_parameter("wo", [P, NH, D], BF16, isOutput=False)
    cos_d = nc.declare_dram_parameter("cos", [P, S], BF16, isOutput=False)
    sin_d = nc.declare_dram_parameter("sin", [P, S], BF16, isOutput=False)
    id_d = nc.declare_dram_parameter("ident", [P, P], BF16, isOutput=False)
    if mode == "causal":
        tm_d = nc.declare_dram_parameter("tmpl", [P, 4, 512], BF16, isOutput=False)
    if mode == "general":
        mask_d = nc.declare_dram_parameter("maskT", [S, S], BF16, isOutput=False)
    out_d = nc.declare_dram_parameter("out", [S, D], BF16, isOutput=True)

    with tile.TileContext(nc) as tc, ExitStack() as octx:
        const = octx.enter_context(tc.tile_pool(name="const", bufs=1))
        wts = octx.enter_context(tc.tile_pool(name="wts", bufs=1))
        xtp = octx.enter_context(tc.tile_pool(name="xt", bufs=3))
        kqv = octx.enter_context(tc.tile_pool(name="kqv", bufs=1))
        rp = octx.enter_context(tc.tile_pool(name="rp", bufs=3))
        pTp = octx.enter_context(tc.tile_pool(name="pT", bufs=33))
        bndp = octx.enter_context(tc.tile_pool(name="bnd", bufs=2))
        lsb = octx.enter_context(tc.tile_pool(name="lsb", bufs=2))
        otnp = octx.enter_context(tc.tile_pool(name="otn", bufs=1))
        osbp = octx.enter_context(tc.tile_pool(name="osb", bufs=2))
        if mode == "general":
            mkp = octx.enter_context(tc.tile_pool(name="mk", bufs=1))

        qwp = octx.enter_context(tc.tile_pool(name="qw", bufs=3, space="PSUM"))
        sp = octx.enter_context(tc.tile_pool(name="sp", bufs=3, space="PSUM"))
        otp = octx.enter_context(tc.tile_pool(name="otp", bufs=1, space="PSUM"))
        lxp = octx.enter_context(tc.tile_pool(name="lx", bufs=1, space="PSUM"))

        # ---- constants / weights (first DMAs feed the first matmuls) ----
        xt_half = {}

        def load_xt(c):
            for half in range(2):
                t = xtp.tile([P, FC, 256], BF16, tag="xt")
                nc.sync.dma_start(t[:], xt_d[:, :, c * TC + half * 256:
                                              c * TC + (half + 1) * 256])
                xt_half[(c, half)] = t

        # split weight loads into fc-chunks so the first projection chain
        # can start as soon as its first slices land
        wq_sb = wts.tile([P, FC, NH, HD], BF16, tag="wq")
        t = xtp.tile([P, FC, 256], BF16, tag="xt")
        xt_half[(0, 0)] = t
        t1 = xtp.tile([P, FC, 256], BF16, tag="xt")
        xt_half[(0, 1)] = t1
        nc.sync.dma_start(wq_sb[:, 0:2], wq_d[:, 0:2])
        nc.sync.dma_start(t[:, 0:4], xt_d[:, 0:4, 0:256])
        nc.sync.dma_start(wq_sb[:, 2:4], wq_d[:, 2:4])
        nc.sync.dma_start(t[:, 4:8], xt_d[:, 4:8, 0:256])
        nc.sync.dma_start(wq_sb[:, 4:8], wq_d[:, 4:8])
        nc.sync.dma_start(t[:, 8:16], xt_d[:, 8:16, 0:256])
        nc.sync.dma_start(wq_sb[:, 8:12], wq_d[:, 8:12])
        nc.sync.dma_start(t1[:, 0:8], xt_d[:, 0:8, 256:512])
        nc.sync.dma_start(wq_sb[:, 12:16], wq_d[:, 12:16])
        nc.sync.dma_start(t1[:, 8:16], xt_d[:, 8:16, 256:512])
        cos_sb = const.tile([P, S], BF16, tag="cos")
        nc.sync.dma_start(cos_sb[:], cos_d[:])
        sin_sb = const.tile([P, S], BF16, tag="sin")
        nc.sync.dma_start(sin_sb[:], sin_d[:])
        wk_sb = wts.tile([P, FC, NH, HD], BF16, tag="wk")
        for g in range(4):
            nc.sync.dma_start(wk_sb[:, 4 * g:4 * g + 4], wk_d[:, 4 * g:4 * g + 4])
        wv_sb = wts.tile([P, FC, DG], BF16, tag="wv")
        nc.sync.dma_start(wv_sb[:], wv_d[:])
        ident = const.tile([P, P], BF16, tag="ident")
        nc.sync.dma_start(ident[:], id_d[:])
        ones_k = const.tile([P, 1], BF16, tag="ones_k")
        nc.vector.memset(ones_k[:], 1.0)
        if mode == "causal":
            tmpl_sb = const.tile([P, 4, 512], BF16, tag="tmpl")
            nc.sync.dma_start(tmpl_sb[:], tm_d[:])
        wo_sb = wts.tile([P, NH, D], BF16, tag="wo")
        nc.sync.dma_start(wo_sb[:], wo_d[:])

        # persistent K^T / Q^T / V tiles
        k_tiles = {}   # (h, c) -> [HD, 512] bf16
        q_tiles = {}   # (h, I) -> [HD, 512] bf16 (bufs=2 per h)
        v_tiles = {}   # kb -> [128tok, NH, HD] bf16
        mk_tiles = {}

        # ---------------- phase-1 emission units ----------------
        def _rope(ps, dest, t0, width):
            csl = cos_sb[:, t0:t0 + width]
            ssl = sin_sb[:, t0:t0 + width]
            ro = rp.tile([P, TC], F32, tag="ro")
            tmp = rp.tile([P, TC], F32, tag="rt")
            nc.vector.tensor_tensor(ro[:, 0:width], ps[:], csl, op=ALU.mult)
            nc.vector.scalar_tensor_tensor(
                tmp[0:64, 0:width], ps[64:128, :], -1.0,
                ssl[0:64, :], op0=ALU.mult, op1=ALU.mult)
            nc.vector.scalar_tensor_tensor(
                tmp[64:128, 0:width], ps[0:64, :], 1.0,
                ssl[64:128, :], op0=ALU.mult, op1=ALU.mult)
            nc.vector.tensor_tensor(dest[:], ro[:, 0:width], tmp[:, 0:width],
                                    op=ALU.add)

        def _qk_dest(c, wsel, h):
            if wsel == 0:
                dest = kqv.tile([HD, TC], BF16, tag=f"q{h}", bufs=2)
                q_tiles[(h, c)] = dest
            else:
                dest = kqv.tile([HD, TC], BF16, tag=f"k{h}_{c}")
                k_tiles[(h, c)] = dest
            return dest

        def qk_unit(c, wsel, h):
            def emit():
                w_sb = wq_sb if wsel == 0 else wk_sb
                ps = qwp.tile([HD, TC], F32, tag="qw")
                for half in range(2):
                    xt_sb = xt_half[(c, half)]
                    sl = slice(half * 256, (half + 1) * 256)
                    for fc in range(FC):
                        nc.tensor.matmul(ps[:, sl], w_sb[:, fc, h, :],
                                         xt_sb[:, fc, :],
                                         start=(fc == 0), stop=(fc == FC - 1))
                dest = _qk_dest(c, wsel, h)
                _rope(ps, dest[:], c * TC, TC)
            return emit, 16 * TC * 0.4166 + 300

        def qk_unit_half(c, wsel, h, half):
            # chunk-0 warmup: per-half units so the first chains don't wait
            # on the second xt DMA
            def emit():
                w_sb = wq_sb if wsel == 0 else wk_sb
                ps = qwp.tile([HD, 256], F32, tag="qw")
                xt_sb = xt_half[(c, half)]
                for fc in range(FC):
                    nc.tensor.matmul(ps[:], w_sb[:, fc, h, :],
                                     xt_sb[:, fc, :],
                                     start=(fc == 0), stop=(fc == FC - 1))
                if half == 0:
                    dest = _qk_dest(c, wsel, h)
                else:
                    dest = q_tiles[(h, c)] if wsel == 0 else k_tiles[(h, c)]
                _rope(ps, dest[:, half * 256:(half + 1) * 256],
                      c * TC + half * 256, 256)
            return emit, 16 * 256 * 0.4166 + 300

        def v_unit(c, pair):
            def emit():
                for tb in (pair * 2, pair * 2 + 1):
                    half, tloc = divmod(tb, 2)
                    xt_sb = xt_half[(c, half)]
                    tsl = slice(tloc * P, (tloc + 1) * P)
                    pv = sp.tile([P, DG], F32, tag="s2")
                    for fc in range(FC):
                        nc.tensor.matmul(pv[:], xt_sb[:, fc, tsl],
                                         wv_sb[:, fc, :],
                                         start=(fc == 0), stop=(fc == FC - 1))
                    kb = c * 4 + tb
                    vt = kqv.tile([P, NH, HD], BF16, tag=f"v{kb}")
                    v_tiles[kb] = vt
                    nc.scalar.copy(vt[:], pv[:])
            return emit, 2 * 16 * DG * 0.4166 + 600

        def p1_units(c):
            units = []
            if c + 1 < KQ:
                units.append((lambda cc=c + 1: load_xt(cc), 100))
            for h in range(NH):
                units.append(qk_unit(c, 0, h))
            for h in range(NH):
                units.append(qk_unit(c, 1, h))
            units.append(v_unit(c, 0))
            units.append(v_unit(c, 1))
            return units

        # ---------------- attention ----------------
        state = {}   # per live step: pT list, l_ps, ot_ps

        def nkb_of(I):
            return 4 * (I + 1) if mode == "causal" else NKB_ALL

        def kb_order(I):
            # masked (diagonal) blocks in the middle: their slower DVE+ACT
            # chains get a head start while the step still ends on fast
            # plain-PSUM exps
            nkb = nkb_of(I)
            if mode == "causal" and I > 0:
                nd = list(range(4 * I))
                half = 2 * I
                return nd[:half] + list(range(4 * I, nkb)) + nd[half:]
            return list(range(nkb))

        def load_masks(I):
            def emit():
                for kb in range(NKB_ALL):
                    t = mkp.tile([P, 512], BF16, tag=f"mk{kb}")
                    nc.sync.dma_start(
                        t[:], mask_d[kb * P:(kb + 1) * P,
                                     I * 512:(I + 1) * 512])
                    mk_tiles[kb] = t
            return emit, 100

        def sc_part(I, h, prev=None):
            # scores+exp for step (I,h); the PREVIOUS step's PV matmuls are
            # woven in per-kb so the PE stays busy while ACT paces the
            # score-slot recycling
            def emit():
                nkb = nkb_of(I)
                qt = q_tiles[(h, I)]
                pT_list = [None] * nkb
                pv_jobs = []
                if prev is not None:
                    pI, ph = prev
                    ps = state[prev]
                    ot = otp.tile([HD, 512], F32, tag="ot")
                    ps["ot"] = ot
                    order1 = kb_order(pI)
                    n1 = len(order1)
                    for i, kb1 in enumerate(order1):
                        def pv(i=i, kb1=kb1, ot=ot, ps=ps, n1=n1, ph=ph):
                            nc.tensor.matmul(ot[:], v_tiles[kb1][:, ph, :],
                                             ps["pT"][kb1][:],
                                             start=(i == 0), stop=(i == n1 - 1))
                        pv_jobs.append(pv)
                npv = len(pv_jobs)
                done_pv = 0
                for j, kb in enumerate(kb_order(I)):
                    st = sp.tile([P, 512], F32, tag="s2")
                    kt = k_tiles[(h, kb // 4)]
                    nc.tensor.matmul(st[:], kt[:, (kb % 4) * P:(kb % 4 + 1) * P],
                                     qt[:], start=True, stop=True)
                    if mode == "causal" and kb >= 4 * I:
                        bnd = bndp.tile([P, 512], F32, tag="bnd")
                        nc.vector.scalar_tensor_tensor(
                            bnd[:], st[:], 0.0, tmpl_sb[:, kb - 4 * I, :],
                            op0=ALU.bypass, op1=ALU.add)
                        src = bnd
                    elif mode == "general":
                        bnd = bndp.tile([P, 512], F32, tag="bnd")
                        nc.vector.scalar_tensor_tensor(
                            bnd[:], st[:], 0.0, mk_tiles[kb][:],
                            op0=ALU.bypass, op1=ALU.add)
                        src = bnd
                    else:
                        src = st
                    pT = pTp.tile([P, 512], BF16, tag="p")
                    nc.scalar.activation(pT[:], src[:], ACTF.Exp, scale=1.0)
                    pT_list[kb] = pT
                    want = (j + 1) * npv // nkb
                    while done_pv < want:
                        pv_jobs[done_pv]()
                        done_pv += 1
                while done_pv < npv:
                    pv_jobs[done_pv]()
                    done_pv += 1
                state[(I, h)] = {"pT": pT_list}
            return emit, (nkb_of(I) + (nkb_of(prev[0]) if prev else 0)) \
                * 512 * 0.4166 + 300

        def tail_part(I, h):
            # softmax denominator chain for step (I,h); emitted one step late
            def emit():
                s = state[(I, h)]
                linv = lsb.tile([P, 4], BF16, tag="linv")
                with nc.allow_low_precision(reason="softmax norm in bf16"):
                    nc.vector.reciprocal(linv[:], s["l"][:])
                lT_ps = lxp.tile([1, 512], BF16, tag="lx")
                for qi in range(4):
                    nc.tensor.transpose(lT_ps[0:1, qi * P:(qi + 1) * P],
                                        linv[:, qi:qi + 1], ident[:])
                lT = lsb.tile([1, 512], BF16, tag="lT")
                nc.scalar.copy(lT[:], lT_ps[:])
                bc = lsb.tile([P, 512], BF16, tag="bc")
                nc.gpsimd.partition_broadcast(bc[:], lT[:])
                otn = otnp.tile([HD, 512], BF16, tag=f"otn{h}")
                nc.vector.tensor_tensor(otn[:], s["ot"][:], bc[:], op=ALU.mult)
                state[(I, h)]["otn"] = otn
            return emit, 4 * P * 0.4166 + 200

        def pv_part(I, h):
            def emit():
                nkb = nkb_of(I)
                s = state[(I, h)]
                ot = otp.tile([HD, 512], F32, tag="ot")
                s["ot"] = ot
                order = kb_order(I)
                for i, kb in enumerate(order):
                    nc.tensor.matmul(ot[:], v_tiles[kb][:, h, :],
                                     s["pT"][kb][:],
                                     start=(i == 0), stop=(i == nkb - 1))
            return emit, nkb_of(I) * 512 * 0.4166 + 300

        def l_part(I, h):
            def emit():
                nkb = nkb_of(I)
                s = state[(I, h)]
                l_ps = lxp.tile([P, 4], F32, tag="lx")
                s["l"] = l_ps
                order = kb_order(I)
                for qi in range(4):
                    for i, kb in enumerate(order):
                        nc.tensor.matmul(l_ps[:, qi:qi + 1],
                                         s["pT"][kb][:, qi * P:(qi + 1) * P],
                                         ones_k[:],
                                         start=(i == 0), stop=(i == nkb - 1))
            return emit, 4 * nkb_of(I) * 4 + 200

        def wo_part(I, sub):
            def emit():
                osb = osbp.tile([P, D], BF16, tag="osb")

                def chain(wps, oc, h_list):
                    for h in h_list:
                        otn = state[(I, h)]["otn"]
                        nc.tensor.matmul(
                            wps[:], otn[:, sub * P:(sub + 1) * P],
                            wo_sb[:, h, oc * 512:(oc + 1) * 512],
                            start=(h == 0), stop=(h == NH - 1))

                def store(oc, wps):
                    if oc % 2 == 0:
                        nc.scalar.copy(osb[:, oc * 512:(oc + 1) * 512], wps[:])
                    else:
                        nc.vector.tensor_copy(osb[:, oc * 512:(oc + 1) * 512],
                                              wps[:])

                # first two chains: emit h0-h2 of both before either h3 so
                # the PE isn't head-of-line blocked on the last head's otn
                w0 = qwp.tile([P, 512], F32, tag="qw")
                chain(w0, 0, range(NH - 1))
                w1 = qwp.tile([P, 512], F32, tag="qw")
                chain(w1, 1, range(NH - 1))
                tb = I * 4 + sub

                def ship(oc):
                    nc.sync.dma_start(
                        out_d[tb * P:(tb + 1) * P, oc * 512:(oc + 1) * 512],
                        osb[:, oc * 512:(oc + 1) * 512])

                chain(w0, 0, [NH - 1])
                store(0, w0)
                ship(0)
                chain(w1, 1, [NH - 1])
                store(1, w1)
                ship(1)
                for oc in (2, 3):
                    wps = qwp.tile([P, 512], F32, tag="qw")
                    chain(wps, oc, range(NH))
                    store(oc, wps)
                    ship(oc)
            return emit, 16 * 512 * 0.4166 + 600

        def free_step(I, h):
            if (I, h) in state:
                del state[(I, h)]

        def att_units(I):
            # per-step parts in pipeline order; tail of (I,h-1) and Wo(I-1)
            # are woven between this step's score and PV emissions.
            units = []
            if mode == "general":
                units.append(load_masks(I))
            for h in range(NH):
                prev = (I, h - 1) if h > 0 else ((I - 1, NH - 1) if I > 0 else None)
                units.append(sc_part(I, h, prev))
                if prev is not None:
                    units.append(tail_part(*prev))
                if I > 0:
                    # spread the previous block's output projection across
                    # the four head-steps to fill PE while exps catch up
                    units.append(wo_part(I - 1, h))
                units.append(l_part(I, h))
            return units

        # ---------------- interleaved emission ----------------
        def run_window(att, p1):
            ta = sum(u[1] for u in att)
            tp = sum(u[1] for u in p1)
            ia = ip = 0
            ca = cp = 0.0
            while ia < len(att) or ip < len(p1):
                # emit from whichever stream is proportionally behind
                if ip >= len(p1):
                    pick_a = True
                elif ia >= len(att):
                    pick_a = False
                else:
                    pick_a = (ca / max(ta, 1)) <= (cp / max(tp, 1))
                if pick_a:
                    att[ia][0]()
                    ca += att[ia][1]
                    ia += 1
                else:
                    p1[ip][0]()
                    cp += p1[ip][1]
                    ip += 1

        if mode == "causal":
            for u in p1_units(0):
                u[0]()
            for I in range(KQ):
                att = att_units(I)
                p1 = p1_units(I + 1) if I + 1 < KQ else []
                run_window(att, p1)
        else:
            # non-causal: every block reads all of K/V -- no interleave
            for c in range(KQ):
                for u in p1_units(c):
                    u[0]()
            for I in range(KQ):
                run_window(att_units(I), [])
        # flush: PV + tail of the last step + Wo(KQ-1)
        pv_part(KQ - 1, NH - 1)[0]()
        tail_part(KQ - 1, NH - 1)[0]()
        for sub in range(4):
            wo_part(KQ - 1, sub)[0]()

    nc.compile()
    return nc


_PROGRAMS = {}


def _get_program(S, mode):
    key = (S, mode)
    if key not in _PROGRAMS:
        _PROGRAMS[key] = build_program(S, mode)
    return _PROGRAMS[key]


def _detect_mode(masks):
    """masks: [B, S, S]. Returns 'zeros' | 'causal' | 'general'."""
    modes = set()
    for mb in masks:
        if not np.any(mb):
            modes.add("zeros")
            continue
        S = mb.shape[0]
        iu = np.triu_indices(S, 1)
        above = mb[iu]
        low_ok = not np.any(np.tril(mb))
        if low_ok and above.size and np.all(above <= -1e8) and \
                np.all(above == above[0]):
            modes.add("causal")
        else:
            modes.add("general")
    if modes == {"zeros"}:
        return "zeros"
    if modes == {"causal"}:
        return "causal"
    return "general"


BF = ml_dtypes.bfloat16


def kernel(hidden_states, attention_mask, position_ids, Wq, Wk, Wv, Wo):
    hidden_states = np.asarray(hidden_states, dtype=np.float32)
    attention_mask = np.asarray(attention_mask, dtype=np.float32)
    position_ids = np.asarray(position_ids)
    Wq = np.asarray(Wq, dtype=np.float32)
    Wk = np.asarray(Wk, dtype=np.float32)
    Wv = np.asarray(Wv, dtype=np.float32)
    Wo = np.asarray(Wo, dtype=np.float32)

    b, S, d = hidden_states.shape
    assert b == B and d == D
    masks = attention_mask.reshape(b, S, S)
    mode = _detect_mode(masks)
    nc = _get_program(S, mode)

    scale = 1.0 / math.sqrt(HD)
    ident = np.eye(P, dtype=np.float32).astype(BF)

    xt_b, cos_b, sin_b, maskT_b = [], [], [], []
    inv_freq = (1.0 / (ROPE_THETA **
                       (np.arange(0, HD, 2, dtype=np.float32) / HD))
                ).astype(np.float32)
    for bi in range(b):
        xt = np.ascontiguousarray(
            hidden_states[bi].T.reshape(FC, P, S).transpose(1, 0, 2)
        ).astype(BF)
        xt_b.append(xt)
        freqs = position_ids[bi].astype(np.float32)[:, None] * inv_freq[None, :]
        emb = np.concatenate([freqs, freqs], axis=-1)  # [S, HD]
        cos_b.append(np.ascontiguousarray(np.cos(emb).T).astype(BF))
        sin_b.append(np.ascontiguousarray(np.sin(emb).T).astype(BF))
        if mode == "general":
            maskT_b.append(np.ascontiguousarray(masks[bi].T).astype(BF))

    if mode == "causal":
        # transposed boundary template: tmplT[p, j, c] = 0 if j*128+p <= c
        pidx = np.arange(P)[:, None, None]
        jidx = np.arange(4)[None, :, None]
        cidx = np.arange(512)[None, None, :]
        tmpl = np.where(jidx * P + pidx <= cidx, 0.0, -1e9
                        ).astype(np.float32).astype(BF)

    in_maps = []
    for c in range(NCORES):
        bi, g = c // 4, c % 4
        gs = slice(g * DG, (g + 1) * DG)
        wq = np.ascontiguousarray(
            (Wq[:, gs] * scale).reshape(FC, P, NH, HD).transpose(1, 0, 2, 3)
        ).astype(BF)
        wk = np.ascontiguousarray(
            Wk[:, gs].reshape(FC, P, NH, HD).transpose(1, 0, 2, 3)).astype(BF)
        wv = np.ascontiguousarray(
            Wv[:, gs].reshape(FC, P, DG).transpose(1, 0, 2)).astype(BF)
        wo = np.ascontiguousarray(
            Wo[gs, :].reshape(NH, P, D).transpose(1, 0, 2)).astype(BF)
        m = dict(xt=xt_b[bi], wq=wq, wk=wk, wv=wv, wo=wo,
                 cos=cos_b[bi], sin=sin_b[bi], ident=ident)
        if mode == "causal":
            m["tmpl"] = tmpl
        if mode == "general":
            m["maskT"] = maskT_b[bi]
        in_maps.append(m)

    import os
    trace = bool(int(os.environ.get("KERNEL_TRACE", "0")))
    res = run_bass_kernel_spmd(nc, in_maps, list(range(NCORES)), trace=trace)
    global LAST_RESULTS
    LAST_RESULTS = res

    out = np.zeros((b, S, D), dtype=np.float32)
    for c in range(NCORES):
        out[c // 4] += res.results[c]["out"].astype(np.float32)
    return out


LAST_RESULTS = None
